# revision 38
# baseline (speedup 1.0000x reference)
"""MCR2 variational loss on 8 Trainium2 NeuronCores.

Math (reference):
  loss_R   = 0.5 * logdet(I + d/(n*eps) * Z.T @ Z)
  loss_Rc  = 0.5 * sum_k(trPi_k * sum_p log1p(d/(trPi_k*eps) * relu(A)_kp)) / n
  loss_reg = 0.5 * sum_k || G_k - Un diag(relu(A)_k) Un.T ||_F^2,
             G_k = Z.T diag(Pi[:,k]) Z
  out = (-(loss_R - loss_Rc - mu*loss_reg), loss_R, loss_Rc, loss_reg)

Fast path (Pi one-hot, which is how setup_inputs builds it): each row
belongs to exactly one class, so the host sorts rows by label and pads
each class to a multiple of 128 rows per core. Every 128-row chunk is
then class-pure: its plain Gram Z_c.T @ Z_c accumulates directly into
that class's PSUM region (start/stop per class). No weighted copies, no
Pi on device, and the full Gram is sum_k G_k on host. Device work drops
to one 128-wide matmul per chunk (~100 chunks/core) and DMA drops to
just Z in fp8 e4m3 — the kernel is HBM-bound at ~1.6 MB/core. Per-class
partial Grams stream back as they finish; the O(k*d^2) epilogue
(slogdet, compress and reg terms) runs on host in float64.
Measured: ~22.6us HW exec (vs 84.4us for the weighted-Gram baseline),
rel err 1.5e-3 (gate 2e-2); ~13.5us of that is fixed NEFF framework
cost (startup barriers/DGE setup ~6.5us in-window + ~7us semaphore-
reset teardown).

Fallback (general Pi weights): the previous weighted-Gram kernel (DVE
builds Pi_k*Z copies, 11-wide matmul per chunk).
"""

import sys

if "/opt/trn_rl_repo" not in sys.path:
    sys.path.insert(0, "/opt/trn_rl_repo")

import ml_dtypes
import numpy as np

import concourse.bacc as bacc
import concourse.mybir as mybir
import concourse.tile as tile
from concourse import bass_utils

# Problem constants (hardcoded per harness contract).
N, D, K = 100000, 128, 10
EPS, MU = 0.5, 1.0
N_CORES = 8

_NC_CACHE = {}

# Fast-path device dtype: fp8 (e4m3) halves DMA vs bf16; rel err ~1.5e-3
# (host-simulated) vs the 2e-2 gate. "bf16" gives ~1e-6 if ever needed.
FAST_DT = "fp8"
_DT_MAP = {
    "bf16": (mybir.dt.bfloat16, ml_dtypes.bfloat16),
    "fp8": (mybir.dt.float8e4, ml_dtypes.float8_e4m3),
}

PS_BANKS = 6  # PSUM banks cycled across class slots (copy never collides
              # with the matmuls of the following slots)


def _dma_groups(M):
    """Input DMA plan: list of (size_chunks, queue). Groups alternate
    between the two HWDGE rings (sync/scalar) in chunk order — the rings
    share the 16 SDMA engines (~280 GB/s aggregate; one ring alone caps
    ~225), and strict alternation keeps delivery roughly in consumption
    order (a bulk transfer on one ring must not starve head groups the
    PE needs first — engines round-robin between rings per packet).
    Few groups: each dma_start costs ~650 ns of issue time on its engine."""
    head = [(2, "sync"), (4, "scalar"), (6, "sync"), (8, "scalar")]
    nh = sum(s for s, _ in head)
    if M <= nh + 16:
        sizes = []
        rem = M
        for s in [2, 4, 6, 8] + [8] * 60:
            take = min(s, rem)
            if take:
                sizes.append(take)
            rem -= take
        return [(s, "sync" if i % 2 == 0 else "scalar")
                for i, s in enumerate(sizes)]
    # Head + middle alternate on the two HWDGE rings in strict chunk
    # order (~260-350 GB/s aggregate). Mixing in gpsimd SWDGE raises raw
    # bandwidth but SWDGE pre-emits its whole descriptor ring, so its
    # packets flood the engines and starve the ordered head chunks the
    # PE needs first — measured net loss at every granularity tried.
    # Exactly 8 input DMAs: the Tile scheduler has 8 DMA-completion sem
    # lanes, so a 9th+ dma_start stalls issuing until an earlier lane
    # recycles — that serialized the old plan's tail delivery.
    head2 = [4, 6, 10, 12]
    rem = M - sum(head2)
    mid_sizes = [rem // 4 + (1 if i < rem % 4 else 0) for i in range(4)]
    plan = head2 + mid_sizes
    return [(s, "sync" if i % 2 == 0 else "scalar")
            for i, s in enumerate(plan)]


def _build_nc_fast(m, dt_name):
    """m: per-class chunks per core (tuple of K ints)."""
    f32 = mybir.dt.float32
    dtb, _ = _DT_MAP[dt_name]
    M = sum(m)
    off = [0]
    for mk in m:
        off.append(off[-1] + mk)

    nc = bacc.Bacc("TRN2", target_bir_lowering=False, debug=False)
    bf16 = mybir.dt.bfloat16
    # Flat layout: each DMA group is a fully contiguous DRAM block of
    # [128, sz*D] (partition-major), so the HBM read side of every DMA is
    # dense — no 26 KB partition stride.
    Zs = nc.dram_tensor("Zs", [128 * M * D], dtb, kind="ExternalInput")
    # Partial Grams ship back as bf16: halves the output HBM writes that
    # contend with the input reads mid-stream and shrinks the drain tail.
    # Error impact ~0.05-0.1% on the summed Grams — an order of magnitude
    # inside the accuracy gate on top of the fp8 input error.
    G = nc.dram_tensor("G", [128, K * D], bf16, kind="ExternalOutput")

    with tile.TileContext(nc) as tc:
        with (
            tc.tile_pool(name="zs", bufs=1) as zpool,
            tc.tile_pool(name="res", bufs=1) as opool,
            tc.tile_pool(name="warm", bufs=1) as warmpool,
            tc.tile_pool(name="ps", bufs=1, space="PSUM") as pspool,
        ):
            ps = [
                pspool.tile([128, 512], f32, name=f"ps{i}")
                for i in range(PS_BANKS)
            ]
            wps = pspool.tile([128, 256], f32, name="wps")

            def ps_slice(k):
                b = k % PS_BANKS
                o = (k // PS_BANKS) * D
                return ps[b][:, o:o + D]

            # PE warmup: dummy matmuls keep the tensor engine busy through
            # the HAM activity window during the ~8us startup preamble so
            # the real matmuls run at the full 2.4 GHz clock. The memset
            # goes on the vector engine (ready ~3.5us; gpsimd would gate
            # the warmups until ~7us). ~24 x 256col ends just before the
            # first data group lands; more would delay the real matmuls
            # (PE FIFO), fewer would let the HAM re-throttle in the gap.
            wsrc = warmpool.tile([128, 256], dtb, name="wsrc")
            nc.vector.memset(wsrc[:], 0.0)

            def warm(n, cols=256):
                for _ in range(n):
                    nc.tensor.matmul(wps[:, 0:cols], wsrc[:, 0:128],
                                     wsrc[:, 0:cols], start=True, stop=True,
                                     skip_group_check=True)

            warm(10)

            # Whole shard lives in one SBUF tile; groups fill disjoint
            # column ranges so matmuls start as soon as their chunk lands.
            zs = zpool.tile([128, M * D], dtb, name="zs")
            c0 = 0
            for sz, qname in _dma_groups(M):
                src = Zs[c0 * 128 * D:(c0 + sz) * 128 * D].rearrange(
                    "(p x) -> p x", p=128
                )
                getattr(nc, qname).dma_start(zs[:, c0 * D:(c0 + sz) * D], src)
                c0 += sz

            out = opool.tile([128, K * D], bf16, name="out")
            # Output batches: flush accumulated copies after these slots;
            # the last batch is a single slot so the drain tail is short.
            flush_after = {3, K - 2, K - 1}
            for k in range(K):
                for c in range(off[k], off[k + 1]):
                    zc = zs[:, c * D:(c + 1) * D]
                    nc.tensor.matmul(ps_slice(k), zc, zc,
                                     start=(c == off[k]),
                                     stop=(c == off[k + 1] - 1))
                # Gap fillers while the input stream is still ramping:
                # keep the PE's HAM activity window dense through the
                # early data stalls so the clock flips to (and stays at)
                # 2.4 GHz ~3.4us in; they run inside the waits, on scratch.
                if k < 5:
                    warm(4, cols=128)
                # DVE copy PSUM -> SBUF right as each slot's accumulation
                # closes (no ACT-table load on the vector engine).
                nc.vector.tensor_copy(out[:, k * D:(k + 1) * D], ps_slice(k))
                if k in flush_after:
                    lo = (max(f for f in flush_after if f < k) + 1
                          if any(f < k for f in flush_after) else 0)
                    # Batched output DMAs on sync: they enter its HWDGE
                    # ring FIFO behind the input descriptors, so they
                    # never compete with input mid-stream (outputs ran
                    # at ~330 GB/s — HBM writes are cheap).
                    nc.sync.dma_start(G[:, lo * D:(k + 1) * D],
                                      out[:, lo * D:(k + 1) * D])

    nc.compile()
    return nc


def _get_nc_fast(m, dt_name):
    key = ("fast", m, dt_name)
    if key not in _NC_CACHE:
        _NC_CACHE[key] = _build_nc_fast(m, dt_name)
    return _NC_CACHE[key]


def _is_one_hot(Pi):
    if Pi.ndim != 2 or Pi.shape[1] != K:
        return False
    return bool(
        np.all((Pi == 0.0) | (Pi == 1.0)) and np.all(Pi.sum(axis=1) == 1.0)
    )


def _prepare_fast(Z, Pi, dt_name=None):
    """Sort rows by class, pad each class to per-core chunk multiples,
    and build per-core [128, M*128] column-major shards."""
    dt_name = dt_name or FAST_DT
    _, dt_np = _DT_MAP[dt_name]
    labels = np.argmax(Pi, axis=1)
    counts = np.bincount(labels, minlength=K).astype(np.int64)
    # chunks per class per core (same on every core -> one SPMD program)
    m = tuple(int(max(1, -(-c // (128 * N_CORES)))) for c in counts)
    M = sum(m)
    off = np.concatenate([[0], np.cumsum(m)])

    order = np.argsort(labels, kind="stable")
    Zc = Z.astype(dt_np)

    shards = [np.zeros((M * 128, D), dt_np) for _ in range(N_CORES)]
    pos = 0
    for k in range(K):
        idx = order[pos:pos + counts[k]]
        pos += counts[k]
        q, r = divmod(int(counts[k]), N_CORES)
        st = 0
        for i in range(N_CORES):
            take = q + (1 if i < r else 0)
            base = off[k] * 128
            shards[i][base:base + take] = Zc[idx[st:st + take]]
            st += take

    sizes = [s for s, _ in _dma_groups(M)]
    in_maps = []
    for s in shards:
        # partition-major view [128, M*D]: column c*D+d = row c*128+p of s
        pm = s.reshape(M, 128, D).transpose(1, 0, 2).reshape(128, M * D)
        # flat group blocks: each DMA group contiguous in DRAM
        blocks = []
        c0 = 0
        for sz in sizes:
            blocks.append(pm[:, c0 * D:(c0 + sz) * D].reshape(-1))
            c0 += sz
        in_maps.append({"Zs": np.ascontiguousarray(np.concatenate(blocks))})
    nc = _get_nc_fast(m, dt_name)
    return nc, in_maps, counts


def _run_device(nc, in_maps, trace=False, tmpdir=None):
    return bass_utils.run_bass_kernel_spmd(
        nc, in_maps, core_ids=list(range(N_CORES)), trace=trace, tmpdir=tmpdir
    )


def _epilogue(Gk, Gram, trPi, A, U):
    """Host epilogue in float64. Gk: [K, D, D], Gram: [D, D]."""
    d_f = float(D)
    n_f = float(N)

    Mat = np.eye(D, dtype=np.float64) + (d_f / (n_f * EPS)) * Gram
    _, logdet = np.linalg.slogdet(Mat)
    loss_R = 0.5 * logdet

    scalar = d_f / (trPi * EPS)
    Ar = np.maximum(A.astype(np.float64), 0.0)          # [K, D]
    logdets = np.log1p(scalar[:, None] * Ar).sum(axis=1)
    loss_Rc = 0.5 * np.sum(logdets * trPi) / n_f

    norms = np.maximum(np.linalg.norm(U, axis=0, keepdims=True), 1e-12)
    Un = (U / norms).astype(np.float64)
    Mk = np.einsum("dp,kp,ep->kde", Un, Ar, Un)
    loss_reg = 0.5 * np.sum((Gk - Mk) ** 2)

    loss_obj = loss_R - loss_Rc - MU * loss_reg
    return (
        np.float32(-loss_obj),
        np.float32(loss_R),
        np.float32(loss_Rc),
        np.float32(loss_reg),
    )


def _kernel_fast(Z, Pi, A, U):
    nc, in_maps, counts = _prepare_fast(Z, Pi)
    res = _run_device(nc, in_maps)
    G_all = np.zeros((128, K * D), np.float64)
    for i in range(N_CORES):
        G_all += np.asarray(res.results[i]["G"], dtype=np.float64)
    Gk = np.stack([G_all[:, k * D:(k + 1) * D] for k in range(K)])
    Gram = Gk.sum(axis=0)
    trPi = counts.astype(np.float64)
    return _epilogue(Gk, Gram, trPi, A, U)


# ---------------------------------------------------------------------------
# General-Pi fallback: weighted-Gram kernel (previous baseline).
# ---------------------------------------------------------------------------

CHUNKS = 98                    # 128-row chunks per core
SHARD = CHUNKS * 128           # 12544 rows per core
NPAD = SHARD * N_CORES         # 100352 (zero-padded; zero rows contribute 0)
GROUP = 7                      # chunks per staged DMA group
NCLS = K + 1                   # 10 masked Grams + 1 full Gram


def _build_nc_general():
    f32 = mybir.dt.float32
    bf16 = mybir.dt.bfloat16

    nc = bacc.Bacc("TRN2", target_bir_lowering=False, debug=False)
    # Per-row payload: [Pi7*Z | Pi8*Z | Pi9*Z | Z_bf16] — classes 7..9
    # weighted on host. One DMA per group feeds everything; a single
    # N=512 matmul over the whole row computes G7, G8, G9 and the Gram.
    ZW = nc.dram_tensor("ZW", [SHARD, 4 * D], bf16, kind="ExternalInput")
    # Pi, host-preprocessed: [p, chunk, class, 2] bf16 with the weight
    # duplicated in the last axis so the DVE reads an aligned [w,w] pair.
    KD = K - 3  # classes 0..6 weighted on DVE; 7..9 host-weighted
    Pb = nc.dram_tensor("Pb", [128, CHUNKS, KD, 2], bf16, kind="ExternalInput")
    G = nc.dram_tensor("G", [D, NCLS * D], f32, kind="ExternalOutput")

    with tile.TileContext(nc) as tc:
        with (
            tc.tile_pool(name="zbf", bufs=6) as zbpool,
            tc.tile_pool(name="wgt", bufs=4) as wpool,
            tc.tile_pool(name="pi", bufs=1) as pipool,
            tc.tile_pool(name="res", bufs=1) as opool,
            tc.tile_pool(name="warm", bufs=1) as warmpool,
            tc.tile_pool(name="ps", bufs=1, space="PSUM") as pspool,
        ):
            psA = pspool.tile([128, 512], f32, name="psA")
            psB = pspool.tile([128, 384], f32, name="psB")
            psC = pspool.tile([128, 512], f32, name="psC")

            # PE warmup: dummy matmuls on scratch data keep the tensor
            # engine busy through the HAM activity window while the first
            # DMAs land, so real matmuls start at the full 2.4 GHz clock.
            wsrc = warmpool.tile([128, 256], bf16, name="wsrc")
            wps = pspool.tile([128, 256], f32, name="wps")
            nc.gpsimd.memset(wsrc[:], 0.0)
            for _ in range(22):
                nc.tensor.matmul(wps[:], wsrc[:, 0:128], wsrc[:], start=True,
                                 stop=True, skip_group_check=True)

            Zr = ZW.rearrange("(c p) d -> p c d", p=128)

            # First chunk's Pi first on the (otherwise idle) gpsimd SWDGE
            # queue so it doesn't serialize behind the Z loads on the sync
            # queue; then the bulk.
            pib = pipool.tile([128, CHUNKS, KD, 2], bf16, name="pib")
            nc.gpsimd.dma_start(pib[:, 0:1], Pb[:, 0:1])
            nc.gpsimd.dma_start(pib[:, 1:8], Pb[:, 1:8])
            nc.gpsimd.dma_start(pib[:, 8:29], Pb[:, 8:29])
            nc.gpsimd.dma_start(pib[:, 29:CHUNKS], Pb[:, 29:CHUNKS])

            # Small first group so compute starts early; tapering last
            # groups to shrink the pipeline drain.
            sizes = [1] + [GROUP] * 12 + [5, 4, 3, 1]
            assert sum(sizes) == CHUNKS

            start_c = 0
            for gi, sz in enumerate(sizes):
                s0 = start_c
                start_c += sz
                zw = zbpool.tile([128, sz, 4 * D], bf16, name="zw", tag="zw")
                nc.sync.dma_start(zw[:], Zr[:, s0:s0 + sz, :])
                zb = zw[:, :, 3 * D:4 * D]

                # Fused weighted-copy for classes 0..7 over the whole group:
                #   wg[p, c, k, 2r+t] = zb[p, c, 2r+t] * pib[p, s0+c, k]
                # bf16 [w,w] pair packing keeps the DVE 2x perf mode.
                wg = wpool.tile([128, sz, KD * D], bf16, name="wg", tag="wg")
                z_bc = zb.unsqueeze(2).broadcast_to([128, sz, KD, D])
                pi_bc = (
                    pib[:, s0:s0 + sz, :, :]
                    .unsqueeze(3)
                    .broadcast_to([128, sz, KD, 64, 2])
                )
                w5 = wg[:, :, 0:KD * D].rearrange(
                    "p c (k r t) -> p c k r t", k=KD, t=2
                )
                z5 = z_bc.rearrange("p c k (r t) -> p c k r t", t=2)
                nc.vector.tensor_mul(w5, z5, pi_bc)

                for c in range(sz):
                    idx = s0 + c
                    first = idx == 0
                    last = idx == CHUNKS - 1
                    zc = zw[:, c, 3 * D:4 * D]
                    w = wg[:, c, :]
                    nc.tensor.matmul(psA[:], zc, w[:, 0:512], start=first, stop=last)
                    nc.tensor.matmul(psB[:], zc, w[:, 512:896], start=first, stop=last)
                    nc.tensor.matmul(psC[:], zc, zw[:, c, :], start=first, stop=last)

                if gi <= 3:
                    # Gap fillers: keep the PE's HAM activity window dense
                    # across the pipeline-fill stalls (they run inside the
                    # wait for the next group's weighted data, on scratch).
                    for _ in range((10, 4, 3, 3)[gi]):
                        nc.tensor.matmul(wps[:], wsrc[:, 0:128], wsrc[:],
                                         start=True, stop=True,
                                         skip_group_check=True)

            out = opool.tile([128, NCLS * D], f32, name="out")
            nc.vector.tensor_copy(out[:, 0:512], psA[:])
            nc.scalar.copy(out[:, 512:896], psB[:])
            nc.sync.dma_start(G[:, 0:896], out[:, 0:896])
            nc.vector.tensor_copy(out[:, 896:1408], psC[:])
            nc.sync.dma_start(G[:, 896:1408], out[:, 896:1408])

    nc.compile()
    return nc


def _get_nc_general():
    if "general" not in _NC_CACHE:
        _NC_CACHE["general"] = _build_nc_general()
    return _NC_CACHE["general"]


def _make_in_maps_general(Z, Pi):
    # Per-row payload [Pi7*Z | Pi8*Z | Pi9*Z | Z], bf16, zero-padded.
    ZWpad = np.zeros((NPAD, 4 * D), ml_dtypes.bfloat16)
    for j in range(3):
        ZWpad[:N, j * D:(j + 1) * D] = (
            Pi[:, K - 3 + j:K - 2 + j] * Z
        ).astype(ml_dtypes.bfloat16)
    ZWpad[:N, 3 * D:4 * D] = Z.astype(ml_dtypes.bfloat16)
    Pipad = np.zeros((NPAD, K), np.float32)
    Pipad[:N] = Pi
    in_maps = []
    for i in range(N_CORES):
        zw = np.ascontiguousarray(ZWpad[i * SHARD:(i + 1) * SHARD])
        pt = (
            Pipad[i * SHARD:(i + 1) * SHARD, 0:K - 3]
            .reshape(CHUNKS, 128, K - 3)
            .transpose(1, 0, 2)
            .astype(ml_dtypes.bfloat16)
        )
        pb = np.ascontiguousarray(np.repeat(pt[..., None], 2, axis=-1))
        in_maps.append({"ZW": zw, "Pb": pb})
    return in_maps


def _kernel_general(Z, Pi, A, U):
    nc = _get_nc_general()
    in_maps = _make_in_maps_general(Z, Pi)
    res = _run_device(nc, in_maps)
    G_all = np.zeros((D, NCLS * D), np.float64)
    for i in range(N_CORES):
        G_all += res.results[i]["G"]
    Gk = np.stack([G_all[:, k * D:(k + 1) * D] for k in range(K)])
    Gram = G_all[:, K * D:(K + 1) * D]
    trPi = Pi.astype(np.float64).sum(axis=0)
    return _epilogue(Gk, Gram, trPi, A, U)


def kernel(Z, Pi, A, U):
    Z = np.asarray(Z, dtype=np.float32)
    Pi = np.asarray(Pi, dtype=np.float32)
    A = np.asarray(A, dtype=np.float32)
    U = np.asarray(U, dtype=np.float32)

    if _is_one_hot(Pi):
        return _kernel_fast(Z, Pi, A, U)
    return _kernel_general(Z, Pi, A, U)
